# revision 60
# baseline (speedup 1.0000x reference)
"""BoundaryLoss kernel for Trainium2 (8 NeuronCores, data-parallel over batch).

Problem: for each (batch, waypoint), find the nearest boundary point (argmin
over N=4096 of euclidean distance), take dot(waypoint - closest_pt,
closest_normal), apply exp_relu, and mean over everything.

Structure (per core: 4 batches x 2 chunks of 128 waypoints = 8 tiles,
processed as 16 half-tiles through a 4-deep PSUM rotation):
  - Centered scores s[w, n] = w.b_n - 0.5||b_n||^2 - 0.5||w||^2 (argmax_n s
    == argmin_n dist; the per-row -0.5||w||^2 shift keeps fold values small
    so rounding near the max is harmless) via float32r matmuls with
    Dekker-split hi/lo operands (exact fp32 reconstruction).
  - Level-1 fold (4096->2048) on PE+ACT: max(s0,s1) = s0 + relu(s1-s0) via
    host-precomputed difference columns, ACT relu, identity-matmul
    accumulation.  Pairs are Morton-adjacent so the f32r re-rounding of the
    relu values is tiny.
  - Each half-tile's 1024 folded values live in their own [128,1024] PSUM
    tile (4 tiles rotate; they must be separate tiles because the framework
    tracks PSUM dependencies per tile).  One DVE a=16 strided reduce per
    half egresses + folds to 64 positions; max8/max_index on the 128
    positions pick the winner.  The whole selection path stays on DVE --
    any cross-engine hop here head-of-line blocks the reduce stream.
  - Tiles 0-5: one indirect DMA per tile gathers the winner's 32 raw
    candidates (1024B rows); the exact fp32 rescore (w4 = [w,-1] against
    [b,hb] and [n,b.n]) runs its mult/add trees on the Pool engine
    (interleaved between gathers; Pool has no compare ops) with an
    argmax-select finish on DVE.
  - Tiles 6,7 skip the gather to shorten the tail: a 0/1 winner mask is
    transposed on PE and used as a permutation matmul against an
    SBUF-resident hi/lo payload table, accumulating the exact fp32 payload
    into PSUM, rescored from there.
  - exp_relu + row-sum on device; host sums the 8 cores' [128] partials.
"""

import numpy as np

import concourse.bass as bass
import concourse.bacc as bacc
import concourse.bass_utils as bass_utils
import concourse.mybir as mybir
from concourse.tile import TileContext

B, W, N, D = 32, 256, 4096, 3
N_CORES = 8
BPC = B // N_CORES          # batches per core = 4
WCHUNKS = W // 128          # waypoint chunks of 128 per batch
TILES = BPC * WCHUNKS       # 8 (batch, wchunk) tiles per core
PAIRS = N // 2              # 2048 Morton pairs per batch
POS = 128                   # fold positions
# Egress split is asymmetric per half-tile: half A (even) folds all 16
# aliases with one DVE a=16 reduce; half B (odd) folds aliases 0-11 on DVE
# (a=12) and aliases 12-15 via ACT copy + Pool tt-max levels, balancing
# DVE/ACT/Pool at ~21us each.
DB_AL = 12                  # half-B aliases egressed by DVE reduce
MB_AL = 4                   # half-B aliases egressed via ACT + Pool
DB_RANGE = 64 * DB_AL       # 768
MB_RANGE = 64 * MB_AL       # 256
NCAND = 32                  # raw candidates per gather row (16 pairs x 2)
ROWF = NCAND * 8            # floats per gather row = 256 (1024B)

# big16 column layout: wa (lhsT columns) first so the very first DMA chunk
# unblocks the matmul stream; then per batch b a contiguous chunk
# [rbd_b (2048 diff-columns) | rb_b (2048 s0-columns)].
WA0 = 0
CHUNK0 = BPC * W            # 1024
BIG16 = CHUNK0 + BPC * 2 * PAIRS

F32 = mybir.dt.float32
F32R = mybir.dt.float32r
I32 = mybir.dt.int32
U32 = mybir.dt.uint32
ALU = mybir.AluOpType
ACTF = mybir.ActivationFunctionType
AX = mybir.AxisListType


def build_bass():
    nc = bacc.Bacc()

    big16 = nc.dram_tensor("big16", [16, BIG16], F32R, kind="ExternalInput")
    big128 = nc.dram_tensor("big128", [128, 128 + TILES * 4], F32R,
                            kind="ExternalInput")
    gsrcs = [nc.dram_tensor(f"gsrc{b}", [POS, ROWF], F32, kind="ExternalInput")
             for b in range(BPC)]
    t3d = nc.dram_tensor("t3", [POS, 2 * ROWF], F32R, kind="ExternalInput")
    res = nc.dram_tensor("res", [128, 1], F32, kind="ExternalOutput")

    with TileContext(nc) as tc:
        with (
            tc.tile_pool(name="const", bufs=1) as cpool,
            tc.tile_pool(name="work", bufs=4) as wpool,
            tc.tile_pool(name="small", bufs=4) as spool,
            tc.tile_pool(name="psum", bufs=1, space="PSUM") as psumpool,
        ):
            # ---- p-state ramp dummy tile ----
            z = cpool.tile([4, 512], mybir.dt.bfloat16)
            nc.vector.memset(z[:], 0.0)

            # ---- input loads: tile-0 critical slices first ----
            sb16 = cpool.tile([16, BIG16], F32R)
            sb128 = cpool.tile([128, 128 + TILES * 4], F32R)
            def rbdv(b):
                c = CHUNK0 + b * 2 * PAIRS
                return sb16[:, c:c + PAIRS]

            def rbv(b):
                c = CHUNK0 + b * 2 * PAIRS + PAIRS
                return sb16[:, c:c + PAIRS]

            # wa + first half of batch-0 diffs: unblocks diffs(0) earliest
            nc.sync.dma_start(out=sb16[:, 0:CHUNK0 + 1024],
                              in_=big16[:, 0:CHUNK0 + 1024])
            nc.sync.dma_start(out=sb128[:], in_=big128[:])
            nc.sync.dma_start(out=sb16[:, CHUNK0 + 1024:CHUNK0 + 2 * PAIRS],
                              in_=big16[:, CHUNK0 + 1024:CHUNK0 + 2 * PAIRS])
            for b in range(1, BPC):
                c = CHUNK0 + b * 2 * PAIRS
                nc.sync.dma_start(out=sb16[:, c:c + 2 * PAIRS],
                                  in_=big16[:, c:c + 2 * PAIRS])
            t3 = cpool.tile([POS, 2 * ROWF], F32R)
            nc.sync.dma_start(out=t3[:], in_=t3d[:])

            wat = sb16[:, WA0:WA0 + BPC * W]
            idt = sb128[:, 0:128]
            w4_all = sb128[:, 128:].bitcast(F32).rearrange(
                "p (t f) -> p t f", f=4)

                        # separate gather-destination tiles per verify group: the
            # framework tracks deps per tile, so one shared tile would
            # serialize later gathers behind earlier verifies' reads
            galls = [cpool.tile([128, 4, ROWF], F32, name="gall03"),
                     cpool.tile([128, 2, ROWF], F32, name="gall45"),
                     cpool.tile([128, 2, ROWF], F32, name="gall67")]

            def gall_view(t0, t1):
                if t1 <= 4:
                    return galls[0][:, t0:t1, :]
                if (t0, t1) == (4, 6):
                    return galls[1][:, :, :]
                return galls[2][:, t0 - 6:t1 - 6, :]

            def gall_dst(t):
                if t < 4:
                    return galls[0][:, t, :]
                return galls[1 + (t - 4) // 2][:, (t - 4) % 2, :]

            dots = cpool.tile([128, TILES], F32)
            i8s = [None] * TILES
            v8s = [None] * TILES

            # PSUM: four independent [128, 1024] tiles (2 banks each) for a
            # 4-deep half-tile rotation.  They MUST be separate tiles: the
            # framework tracks PSUM dependencies per tile, so sub-ranges of
            # one big tile would falsely serialize against each other.
            P = [psumpool.tile([128, 1024], F32, tag=f"P{i}", name=f"P{i}")
                 for i in range(4)]
            X = P[0]

            # ---- PE warm-up during input DMA ----
            for k in range(4):
                nc.tensor.matmul(out=X[0:1, 0:512], lhsT=z[:, 0:1],
                                 rhs=z[:], start=True, stop=True)
            nc.tensor.matmul(out=X[0:1, 0:2], lhsT=wat[:, 0:1],
                             rhs=wat[:, 0:2], start=True, stop=True)
            nc.tensor.matmul(out=X[0:1, 2:4], lhsT=idt[:, 0:1],
                             rhs=idt[:, 0:2], start=True, stop=True)
            nc.tensor.matmul(out=X[0:1, 4:6], lhsT=wat[:, 0:1],
                             rhs=rbdv(0)[:, 0:2], start=True, stop=True)
            nc.tensor.matmul(out=X[0:1, 6:8], lhsT=wat[:, 0:1],
                             rhs=rbv(0)[:, 0:2], start=True, stop=True)
            HALVES = 2 * TILES
            avs = [None] * HALVES
            mds = [None] * TILES
            fds = [None] * TILES
            fms = [None] * TILES

            def lhsT_of(t):
                b, wc = divmod(t, WCHUNKS)
                return wat[:, b * W + 128 * wc:b * W + 128 * (wc + 1)]

            def region(u):
                # half-tile u rotates through the four PSUM tiles
                return P[u % 4], 0

            def fr_diff(u):
                # diff = s0 - s1 into this half's region; a = relu(-diff).
                t, h = divmod(u, 2)
                b = t // WCHUNKS
                lhsT = lhsT_of(t)
                y, off = region(u)
                a = wpool.tile([128, 1024], F32R, tag="a", name=f"a{u}", bufs=6)
                for k in range(2):
                    sl = slice(off + 512 * k, off + 512 * (k + 1))
                    nc.tensor.matmul(out=y[:, sl], lhsT=lhsT,
                                     rhs=rbdv(b)[:, 1024 * h + 512 * k:
                                                 1024 * h + 512 * (k + 1)],
                                     start=True, stop=True)
                nc.scalar.activation(out=a[:], in_=y[:, off:off + 1024],
                                     func=ACTF.Relu, scale=-1.0)
                avs[u] = a

            def fr_main(u):
                # s0 overwrites the region; identity matmuls add the relu.
                t, h = divmod(u, 2)
                b = t // WCHUNKS
                lhsT = lhsT_of(t)
                y, off = region(u)
                a = avs[u]
                for k in range(2):
                    sl = slice(off + 512 * k, off + 512 * (k + 1))
                    nc.tensor.matmul(out=y[:, sl], lhsT=lhsT,
                                     rhs=rbv(b)[:, 1024 * h + 512 * k:
                                                1024 * h + 512 * (k + 1)],
                                     start=True, stop=False)
                    nc.tensor.matmul(out=y[:, sl], lhsT=idt[:],
                                     rhs=a[:, 512 * k:512 * (k + 1)],
                                     start=False, stop=True)

            def egress(u):
                # all 16 aliases of position 64h+q' sit in the region's cols
                # [16q', 16q'+16); one DVE a=16 strided reduce per half
                # egresses + folds the region.  Keeping the whole selection
                # path on DVE avoids any cross-engine head-of-line blocking.
                t, h = divmod(u, 2)
                if h == 0:
                    f = spool.tile([128, POS], F32, tag="f", name=f"f{t}",
                                   bufs=4)
                    fds[t] = f
                f = fds[t]
                y, _ = region(u)
                nc.vector.tensor_reduce(
                    out=f[:, 64 * h:64 * h + 64],
                    in_=y[:].rearrange("p (q a) -> p q a", a=16),
                    axis=AX.X, op=ALU.max)

            def sel_block(t):
                # find the winning position among the 128 folded values
                f = fds[t]
                v8 = spool.tile([128, 8], F32, tag="v8", bufs=6,
                                name=f"v8_{t}")
                nc.vector.max(out=v8[:], in_=f[:])
                v8s[t] = v8
                i8 = spool.tile([128, 8], U32, tag="i8", bufs=6,
                                name=f"i8_{t}")
                nc.vector.max_index(out=i8[:], in_max=v8[:], in_values=f[:])
                i8s[t] = i8

            def gather(t):
                # one indirect DMA per tile, fired as soon as its winner
                # index is known ([128,1] offsets, column 0 of the i8 tile)
                b = t // WCHUNKS
                nc.gpsimd.indirect_dma_start(
                    out=gall_dst(t), out_offset=None,
                    in_=gsrcs[b][:],
                    in_offset=bass.IndirectOffsetOnAxis(
                        ap=i8s[t][:, 0:1].bitcast(I32), axis=0))

            vp_groups = []
            vst = {}

            def verify_pool(t0, t1):
                # heavy verify ops (mult/add only -- Pool has no compare or
                # max ALU support) on the Pool engine; appended to vp_groups
                # as closures so the emission loop can interleave them
                # between gathers without clogging the Pool queue.  The
                # compare/select finish runs on DVE later (verify_fin).
                n = t1 - t0
                g = gall_view(t0, t1).rearrange(
                    "p t (c f) -> p t c f", f=8)
                wpv = w4_all[:, t0:t1, :].unsqueeze(2).broadcast_to(
                    [128, n, NCAND, 4])
                gp = nc.gpsimd

                def ptile(nm, shape):
                    return cpool.tile(shape, F32, tag=f"{nm}{t0}",
                                      name=f"{nm}{t0}")

                spr = ptile("spr", [128, n, NCAND, 4])
                sc1 = ptile("sc1", [128, n, NCAND, 2])
                sc = ptile("sc", [128, n, NCAND])
                dpr = ptile("dpr", [128, n, NCAND, 4])
                dt1 = ptile("dt1", [128, n, NCAND, 2])
                dt = ptile("dt", [128, n, NCAND])
                vst[t0] = (sc, dt)
                vp_groups.append(lambda: gp.tensor_tensor(
                    out=spr[:], in0=wpv, in1=g[:, :, :, 0:4], op=ALU.mult))
                vp_groups.append(lambda: gp.tensor_tensor(
                    out=sc1[:], in0=spr[:, :, :, 0:2], in1=spr[:, :, :, 2:4],
                    op=ALU.add))
                vp_groups.append(lambda: gp.tensor_tensor(
                    out=sc[:], in0=sc1[:, :, :, 0], in1=sc1[:, :, :, 1],
                    op=ALU.add))
                vp_groups.append(lambda: gp.tensor_tensor(
                    out=dpr[:], in0=wpv, in1=g[:, :, :, 4:8], op=ALU.mult))
                vp_groups.append(lambda: gp.tensor_tensor(
                    out=dt1[:], in0=dpr[:, :, :, 0:2], in1=dpr[:, :, :, 2:4],
                    op=ALU.add))
                vp_groups.append(lambda: gp.tensor_tensor(
                    out=dt[:], in0=dt1[:, :, :, 0], in1=dt1[:, :, :, 1],
                    op=ALU.add))

            def vp_drain(k):
                for _ in range(k):
                    if vp_groups:
                        vp_groups.pop(0)()

            def verify_fin(t0, t1):
                # DVE finish: argmax-select the candidate and emit the dot
                n = t1 - t0
                sc, dt = vst[t0]
                mx = cpool.tile([128, n, 1], F32, tag=f"fmx{t0}",
                                name=f"fmx{t0}")
                nc.vector.tensor_reduce(out=mx[:], in_=sc[:], axis=AX.X,
                                        op=ALU.max)
                msk = cpool.tile([128, n, NCAND], F32, tag=f"fmk{t0}",
                                 name=f"fmk{t0}")
                nc.vector.tensor_tensor(
                    out=msk[:], in0=sc[:],
                    in1=mx[:].broadcast_to([128, n, NCAND]), op=ALU.is_ge)
                sel = cpool.tile([128, n, NCAND], F32, tag=f"fsl{t0}",
                                 name=f"fsl{t0}")
                nc.vector.tensor_tensor(out=sel[:], in0=msk[:], in1=dt[:],
                                        op=ALU.mult)
                nc.vector.tensor_reduce(out=dots[:, t0:t1], in_=sel[:],
                                        axis=AX.X, op=ALU.add)

            def verify(t0, t1, tail=True):
                # exact rescore of the 32 gathered candidates per waypoint on
                # DVE (tail path, latency-optimized): sc = w.b - hb ranks
                # candidates; dt = w.n - b.n is the output dot; the argmax
                # row is selected by an is_ge mask (exact fp32 ties are
                # measure-zero).
                n = t1 - t0
                g = gall_view(t0, t1).rearrange("p t (c f) -> p t c f", f=8)
                wpv = w4_all[:, t0:t1, :].unsqueeze(2).broadcast_to(
                    [128, n, NCAND, 4])
                spr = cpool.tile([128, n, NCAND, 4], F32, tag=f"spr{t0}",
                                 name=f"spr{t0}")
                nc.vector.tensor_tensor(out=spr[:], in0=wpv,
                                        in1=g[:, :, :, 0:4], op=ALU.mult)
                sc = cpool.tile([128, n, NCAND], F32, tag=f"sc{t0}",
                                name=f"sc{t0}")
                nc.vector.tensor_reduce(out=sc[:], in_=spr[:],
                                        axis=AX.X, op=ALU.add)
                dpr = cpool.tile([128, n, NCAND, 4], F32, tag=f"dpr{t0}",
                                 name=f"dpr{t0}")
                nc.vector.tensor_tensor(out=dpr[:], in0=wpv,
                                        in1=g[:, :, :, 4:8], op=ALU.mult)
                dt = cpool.tile([128, n, NCAND], F32, tag=f"dt{t0}",
                                name=f"dt{t0}")
                nc.vector.tensor_reduce(out=dt[:], in_=dpr[:],
                                        axis=AX.X, op=ALU.add)
                mx = cpool.tile([128, n, 1], F32, tag=f"mx{t0}",
                                name=f"mx{t0}")
                nc.vector.tensor_reduce(out=mx[:], in_=sc[:], axis=AX.X,
                                        op=ALU.max)
                msk = cpool.tile([128, n, NCAND], F32, tag=f"msk{t0}",
                                 name=f"msk{t0}")
                nc.vector.tensor_tensor(
                    out=msk[:], in0=sc[:],
                    in1=mx[:].broadcast_to([128, n, NCAND]), op=ALU.is_ge)
                sel = cpool.tile([128, n, NCAND], F32, tag=f"sel{t0}",
                                 name=f"sel{t0}")
                nc.vector.tensor_tensor(out=sel[:], in0=msk[:], in1=dt[:],
                                        op=ALU.mult)
                nc.vector.tensor_reduce(out=dots[:, t0:t1], in_=sel[:],
                                        axis=AX.X, op=ALU.add)

            def exp_tail(t0, t1):
                # exp_relu(dots[t0:t1]) -> em1[t0:t1]
                nc.scalar.activation(out=e[:, t0:t1], in_=dots[:, t0:t1],
                                     func=ACTF.Exp, scale=0.5)
                nc.vector.tensor_scalar(out=em1[:, t0:t1], in0=e[:, t0:t1],
                                        scalar1=-1.0, scalar2=None,
                                        op0=ALU.add)
                nc.vector.tensor_scalar(out=gmask[:, t0:t1],
                                        in0=dots[:, t0:t1], scalar1=0.0,
                                        scalar2=None, op0=ALU.is_gt)
                nc.vector.copy_predicated(em1[:, t0:t1], gmask[:, t0:t1],
                                          dots[:, t0:t1])

            e = cpool.tile([128, TILES], F32)
            em1 = cpool.tile([128, TILES], F32)
            gmask = cpool.tile([128, TILES], U32)

            # ---- software-pipelined main loop over 16 half-tiles with a
            # 4-deep PSUM region rotation; diffs+relu run two halves ahead
            # so PE never head-of-line blocks on the relu; selection for
            # tile t is emitted one tile later so the DVE queue never waits
            # on the slower ACT+Pool egress path ----
            # Emission schedule: sel(t) goes out half a tile late (after
            # egress_a(t+1)) so the DVE queue never waits on the Pool
            # combine; verify(0,4) runs as two 2-tile chunks whose slow Pool
            # ops are placed mid-stream and whose DVE ops are deferred past
            # the selection chain.
            fr_diff(0)
            fr_diff(1)
            for u in range(HALVES):
                fr_main(u)
                if u + 2 < HALVES:
                    fr_diff(u + 2)
                egress(u)
                if u % 2 == 1:
                    t = u // 2
                    sel_block(t)
                    if t < TILES - 2:
                        gather(t)
                    if t == 4:
                        verify_pool(0, 4)
                    if t == 6:
                        verify_pool(4, 6)
                else:
                    vp_drain(2)
            vp_drain(12)
            verify_fin(0, 4)
            verify_fin(4, 6)
            exp_tail(0, 4)
            exp_tail(4, 6)
            # tiles 6,7 (batch 3): no gather -- build a 0/1 position mask,
            # transpose it on PE, and use it as a permutation matmul against
            # the SBUF payload table (hi+lo accumulated exactly); rescore
            # from PSUM.
            def trick_verify(t, preg):
                msk2 = cpool.tile([128, POS], F32R, name=f"msk2_{t}")
                nc.vector.tensor_scalar(out=msk2[:], in0=fds[t][:],
                                        scalar1=v8s[t][:, 0:1], scalar2=None,
                                        op0=ALU.is_ge)
                nc.tensor.transpose(out=preg[:, 0:128].bitcast(F32R),
                                    in_=msk2[:], identity=idt[:])
                mT = cpool.tile([128, POS], F32R, name=f"mT{t}")
                nc.scalar.copy(out=mT[:], in_=preg[:, 0:128])
                g7 = preg[:, 256:256 + ROWF]
                nc.tensor.matmul(out=g7, lhsT=mT[:], rhs=t3[:, 0:ROWF],
                                 start=True, stop=False)
                nc.tensor.matmul(out=g7, lhsT=mT[:],
                                 rhs=t3[:, ROWF:2 * ROWF],
                                 start=False, stop=True)
                gv = g7.bitcast(F32).rearrange("p (c f) -> p c f", f=8)
                wpv7 = w4_all[:, t:t + 1, :].rearrange(
                    "p o f -> p (o f)").unsqueeze(1).broadcast_to(
                    [128, NCAND, 4])
                spr7 = cpool.tile([128, NCAND, 4], F32, name=f"spr7_{t}")
                nc.vector.tensor_tensor(out=spr7[:], in0=wpv7,
                                        in1=gv[:, :, 0:4], op=ALU.mult)
                sc7 = cpool.tile([128, NCAND], F32, name=f"sc7_{t}")
                nc.vector.tensor_reduce(out=sc7[:], in_=spr7[:], axis=AX.X,
                                        op=ALU.add)
                dpr7 = cpool.tile([128, NCAND, 4], F32, name=f"dpr7_{t}")
                nc.vector.tensor_tensor(out=dpr7[:], in0=wpv7,
                                        in1=gv[:, :, 4:8], op=ALU.mult)
                dt7 = cpool.tile([128, NCAND], F32, name=f"dt7_{t}")
                nc.vector.tensor_reduce(out=dt7[:], in_=dpr7[:], axis=AX.X,
                                        op=ALU.add)
                mx7 = cpool.tile([128, 1], F32, name=f"mx7_{t}")
                nc.vector.tensor_reduce(out=mx7[:], in_=sc7[:], axis=AX.X,
                                        op=ALU.max)
                seldt7 = cpool.tile([128, NCAND], F32, name=f"seldt7_{t}")
                nc.vector.scalar_tensor_tensor(
                    out=seldt7[:], in0=sc7[:], scalar=mx7[:, 0:1],
                    in1=dt7[:], op0=ALU.is_ge, op1=ALU.mult)
                nc.vector.tensor_reduce(out=dots[:, t:t + 1], in_=seldt7[:],
                                        axis=AX.X, op=ALU.add)

            trick_verify(TILES - 2, P[0])
            trick_verify(TILES - 1, P[2])
            exp_tail(6, 8)
            sums = cpool.tile([128, 1], F32)
            nc.vector.reduce_sum(out=sums[:], in_=em1[:], axis=AX.X)
            nc.sync.dma_start(out=res[:], in_=sums[:])

    nc.finalize()
    return nc


_NC_CACHE = None


def _get_nc():
    global _NC_CACHE
    if _NC_CACHE is None:
        _NC_CACHE = build_bass()
    return _NC_CACHE


def _split12(x):
    """Split fp32 array into hi (top 12 mantissa bits, f32r-exact) + lo."""
    x = np.asarray(x, dtype=np.float32)
    c = np.float32((1 << 12) + 1)
    t = (c * x).astype(np.float32)
    hi = (t - (t - x).astype(np.float32)).astype(np.float32)
    lo = (x - hi).astype(np.float32)
    return hi, lo


def _morton_order(bp):
    lo = bp.min(0)
    span = bp.max(0) - lo + 1e-9
    q = np.floor((bp - lo) / span * 31.999).astype(np.int64)
    code = np.zeros(bp.shape[0], dtype=np.int64)
    for i in range(5):
        for d in range(3):
            code |= ((q[:, d] >> i) & 1) << (3 * i + d)
    return np.argsort(code, kind="stable")


def _pack16(bT, sq, row15):
    """[16, cols] f32 block from [3, cols] coords + [3, cols] sq-parts."""
    bh, bl = _split12(bT)
    sh, sl = _split12(sq)
    out = np.zeros((16, bT.shape[1]), dtype=np.float32)
    out[0:3] = bh
    out[3:6] = bl
    out[6:9] = bh
    out[9:12] = sh
    out[12:15] = sl
    out[15] = row15
    return out


# rb/rbd (== PSUM) column -> pair index: identity.  Position q = c//16,
# alias k = c%16 -> pair 16q+k, for both halves.
def _tau():
    return np.arange(PAIRS, dtype=np.int64)


_TAU = _tau()


def make_in_maps(waypoints, boundarypoints, boundarynormals):
    waypoints = np.ascontiguousarray(waypoints, dtype=np.float32)
    boundarypoints = np.ascontiguousarray(boundarypoints, dtype=np.float32)
    boundarynormals = np.ascontiguousarray(boundarynormals, dtype=np.float32)
    in_maps = []
    for c in range(N_CORES):
        sl = slice(c * BPC, (c + 1) * BPC)
        wp_c = waypoints[sl]                      # [4, 256, 3]
        bp_c = boundarypoints[sl]                 # [4, 4096, 3]
        nrm_c = boundarynormals[sl]               # [4, 4096, 3]

        big16 = np.zeros((16, BIG16), dtype=np.float32)
        # lhsT rows: wh, wh, wl, -0.5 x6, -0.5||w||^2
        wT = wp_c.transpose(0, 2, 1).reshape(BPC, D, W)
        for b in range(BPC):
            wh, wl = _split12(wT[b])
            blk = slice(WA0 + b * W, WA0 + (b + 1) * W)
            big16[0:3, blk] = wh
            big16[3:6, blk] = wh
            big16[6:9, blk] = wl
            big16[9:15, blk] = -0.5
            big16[15, blk] = -0.5 * (wT[b] * wT[b]).sum(0, dtype=np.float32)

        gsrcs = {}
        for b in range(BPC):
            order = _morton_order(bp_c[b])
            s0 = bp_c[b][order[0::2]]             # [2048, 3] pair members 0
            s1 = bp_c[b][order[1::2]]             # [2048, 3] pair members 1
            s0sq = (s0 * s0).astype(np.float32)
            s1sq = (s1 * s1).astype(np.float32)
            # rbd then rb in the batch chunk (tau order)
            c0 = CHUNK0 + b * 2 * PAIRS
            db = (s0 - s1).astype(np.float32)
            ds = (s0sq - s1sq).astype(np.float32)
            big16[:, c0:c0 + PAIRS] = _pack16(
                db[_TAU].T.astype(np.float32), ds[_TAU].T.astype(np.float32),
                0.0)
            big16[:, c0 + PAIRS:c0 + 2 * PAIRS] = _pack16(
                s0[_TAU].T.astype(np.float32), s0sq[_TAU].T.astype(np.float32),
                1.0)
            # gather table: row q = 16 pairs x 2 members x
            # [bx by bz hb nx ny nz b.n]
            g = np.empty((POS, NCAND, 8), dtype=np.float32)
            for k in range(16):
                pr = 16 * np.arange(POS) + k      # pair indices of alias k
                for m in range(2):
                    pts = bp_c[b][order[2 * pr + m]]
                    nrs = nrm_c[b][order[2 * pr + m]]
                    g[:, 2 * k + m, 0:3] = pts
                    g[:, 2 * k + m, 3] = 0.5 * (pts * pts).sum(
                        1, dtype=np.float32)
                    g[:, 2 * k + m, 4:7] = nrs
                    g[:, 2 * k + m, 7] = (pts * nrs).sum(1, dtype=np.float32)
            gsrcs[f"gsrc{b}"] = np.ascontiguousarray(
                g.reshape(POS, ROWF))

        big128 = np.zeros((128, 128 + TILES * 4), dtype=np.float32)
        big128[:, 0:128] = np.eye(128, dtype=np.float32)
        for t in range(TILES):
            b, wc = divmod(t, WCHUNKS)
            big128[:, 128 + t * 4:128 + t * 4 + 3] = \
                wp_c[b, 128 * wc:128 * (wc + 1), :]
            big128[:, 128 + t * 4 + 3] = -1.0

        g3 = gsrcs[f"gsrc{BPC - 1}"]
        hi, lo = _split12(g3)
        t3 = np.concatenate([hi, lo], axis=1)
        in_maps.append({
            "big16": np.ascontiguousarray(big16),
            "big128": np.ascontiguousarray(big128),
            "t3": np.ascontiguousarray(t3),
            **gsrcs,
        })
    return in_maps


def run_on_device(waypoints, boundarypoints, boundarynormals, trace=False):
    nc = _get_nc()
    in_maps = make_in_maps(waypoints, boundarypoints, boundarynormals)
    out = bass_utils.run_bass_kernel_spmd(
        nc, in_maps, core_ids=list(range(N_CORES)), trace=trace)
    total = np.float64(0.0)
    for r in out.results:
        total += np.sum(r["res"], dtype=np.float64)
    value = np.float32(total / (B * W))
    return value, out


def kernel(waypoints, boundarypoints, boundarynormals):
    value, _ = run_on_device(waypoints, boundarypoints, boundarynormals)
    return np.asarray(value, dtype=np.float32)
